# revision 2
# baseline (speedup 1.0000x reference)
"""Multi-head attention (RoPE + causal SDPA) on 8 Trainium2 NeuronCores.

Sharding: tensor-parallel over heads for QKV+attention (2 heads/core),
then an on-device AllToAll reshards from head-split to sequence-split,
and each core computes its sequence slice of the output projection with
the full wo. Host side only slices/transposes/concatenates.

Layout trick: everything is computed "transposed" (feature dims on SBUF
partitions, sequence on the free axis), so no on-chip transposes are
needed anywhere:
  Qt/Kt  (128=2*64 head dims, S)  = W_slice @ x.T    (lhsT=W_sliceT chunks)
  scores (128 kpos, 512 q)        = Kt_blk.T-style matmul (lhsT=Kt blk)
  P.T    = exp(scores * 1/8)      on ScalarE, softmax sum via an extra
                                   ones-column appended to V (lhsT=[V|1])
  O.T    (64+1, 512 q)            accumulated over k blocks in PSUM
  out.T  (1024 e, rows)           = woT chunks.T @ O_full.T
"""
import sys, os
if '/opt/trn_rl_repo' not in sys.path:
    sys.path.insert(0, '/opt/trn_rl_repo')
os.environ.setdefault('MYCRO_LOCAL_CACHE', '1')

from contextlib import ExitStack

import numpy as np
import ml_dtypes

import concourse.bass as bass
import concourse.tile as tile
from concourse import bacc, mybir
from concourse.bass_utils import run_bass_kernel_spmd

BF16 = ml_dtypes.bfloat16
NC = 8           # cores
B = 2            # batch
D = 1024         # model dim
H = 16           # heads
HD = 64          # head dim
HPC = H // NC    # heads per core = 2
DPC = HPC * HD   # head dims per core = 128
ROPE_BASE = 10000.0
QT = 512         # q tile (free axis of score matmuls)
KB = 128         # k block (partition axis of score matmuls)

F32 = mybir.dt.float32
BF = mybir.dt.bfloat16


def build_nc(S):
    """Build+compile the SPMD Bass module for sequence length S."""
    R = B * S              # total (b,s) rows
    RPC = R // NC          # rows per core after AllToAll
    NQT = S // QT          # q tiles per (b,h)
    NVT = S // KB          # V tiles (of 128 kpos) per (b,h)
    DCH = D // 128         # contraction chunks (= 8)

    nc = bacc.Bacc(num_devices=NC)

    xT = nc.declare_dram_parameter("xT", [D, R], BF, isOutput=False)
    wqT = nc.declare_dram_parameter("wqT", [D, DPC], BF, isOutput=False)
    wkT = nc.declare_dram_parameter("wkT", [D, DPC], BF, isOutput=False)
    wvT = nc.declare_dram_parameter("wvT", [D, DPC], BF, isOutput=False)
    woT = nc.declare_dram_parameter("woT", [D, D], BF, isOutput=False)
    cosT = nc.declare_dram_parameter("cosT", [128, S], BF, isOutput=False)
    sinT = nc.declare_dram_parameter("sinT", [128, S], BF, isOutput=False)
    maskT = nc.declare_dram_parameter("maskT", [4, 128, QT], BF, isOutput=False)
    out = nc.declare_dram_parameter("out", [D, RPC], F32, isOutput=True)

    a2a_in = nc.dram_tensor("a2a_in", [NC, 128, RPC], BF)
    a2a_out = nc.dram_tensor("a2a_out", [NC, 128, RPC], BF)

    ctx = ExitStack()
    with ctx:
        tc = ctx.enter_context(tile.TileContext(nc))

        consts = ctx.enter_context(tc.tile_pool(name="consts", bufs=1))
        xpool = ctx.enter_context(tc.tile_pool(name="x", bufs=2 * DCH))
        pQt = ctx.enter_context(tc.tile_pool(name="qt", bufs=2))
        pKt = ctx.enter_context(tc.tile_pool(name="kt", bufs=2))
        pV = ctx.enter_context(tc.tile_pool(name="v", bufs=2))
        pO = ctx.enter_context(tc.tile_pool(name="oall", bufs=1))
        ptmp = ctx.enter_context(tc.tile_pool(name="tmp", bufs=4))
        ppt = ctx.enter_context(tc.tile_pool(name="pt", bufs=4))
        pnorm = ctx.enter_context(tc.tile_pool(name="norm", bufs=2))
        pog = ctx.enter_context(tc.tile_pool(name="og", bufs=NC))
        posb = ctx.enter_context(tc.tile_pool(name="osb", bufs=2))

        ppA = ctx.enter_context(tc.tile_pool(name="ppA", bufs=3, space="PSUM"))
        pps = ctx.enter_context(tc.tile_pool(name="pps", bufs=2, space="PSUM"))
        ppo = ctx.enter_context(tc.tile_pool(name="ppo", bufs=2, space="PSUM"))
        ppb = ctx.enter_context(tc.tile_pool(name="ppb", bufs=1, space="PSUM"))

        # ---- constants into SBUF ----
        def load_w(dram, ncols):
            ts = []
            for d in range(DCH):
                t = consts.tile([128, ncols], BF, tag=f"w{dram.name}{d}")
                nc.sync.dma_start(out=t[:], in_=dram[d * 128:(d + 1) * 128, :])
                ts.append(t)
            return ts

        wq_sb = load_w(wqT, DPC)
        wk_sb = load_w(wkT, DPC)
        wv_sb = load_w(wvT, DPC)
        wo_sb = load_w(woT, D)

        cos_sb = consts.tile([128, S], BF, tag="cos")
        nc.sync.dma_start(out=cos_sb[:], in_=cosT[:, :])
        sin_sb = consts.tile([128, S], BF, tag="sin")
        nc.sync.dma_start(out=sin_sb[:], in_=sinT[:, :])
        mask_sb = []
        for j in range(4):
            m = consts.tile([128, QT], BF, tag=f"mask{j}")
            nc.sync.dma_start(out=m[:], in_=maskT[j, :, :])
            mask_sb.append(m)
        ones_sb = consts.tile([1, HD], F32, tag="ones")
        nc.vector.memset(ones_sb[:], 1.0)

        # ---- phase A: projections + RoPE, per batch ----
        qt_t, kt_t, v_t = [], [], []
        for b in range(B):
            xts = []
            for d in range(DCH):
                t = xpool.tile([128, S], BF, tag="xt")
                nc.sync.dma_start(out=t[:], in_=xT[d * 128:(d + 1) * 128,
                                                   b * S:(b + 1) * S])
                xts.append(t)

            def project_rope(w_sb, dest_pool, tag):
                dest = dest_pool.tile([128, S], BF, tag=tag)
                for st in range(NQT):
                    ps = ppA.tile([128, QT], F32, tag="pA")
                    for d in range(DCH):
                        nc.tensor.matmul(
                            ps[:], w_sb[d][:], xts[d][:, st * QT:(st + 1) * QT],
                            start=(d == 0), stop=(d == DCH - 1))
                    c0 = st * QT
                    raw = ptmp.tile([128, QT], BF, tag="raw")
                    nc.scalar.copy(raw[:], ps[:])
                    tcos = ptmp.tile([128, QT], BF, tag="tcos")
                    nc.vector.tensor_tensor(
                        tcos[:], raw[:], cos_sb[:, c0:c0 + QT],
                        mybir.AluOpType.mult)
                    trot = ptmp.tile([128, QT], BF, tag="trot")
                    for g in range(4):
                        o0 = g * 32
                        i0 = (g * 32 + 32) if g % 2 == 0 else (g * 32 - 32)
                        nc.vector.tensor_copy(
                            trot[o0:o0 + 32, :], raw[i0:i0 + 32, :])
                    nc.vector.tensor_tensor(
                        trot[:], trot[:], sin_sb[:, c0:c0 + QT],
                        mybir.AluOpType.mult)
                    nc.vector.tensor_tensor(
                        dest[:, c0:c0 + QT], tcos[:], trot[:],
                        mybir.AluOpType.add)
                return dest

            qt_t.append(project_rope(wq_sb, pQt, "qt"))
            kt_t.append(project_rope(wk_sb, pKt, "kt"))

            # V: natural (kpos on partitions), with a ones column appended
            # per head: vtile[:, t, 0:65] = [V_headA | 1], [:, t, 65:130].
            vt = pV.tile([128, NVT, 130], BF, tag="vt")
            nc.vector.memset(vt[:], 1.0)
            for t in range(NVT):
                ps = ppA.tile([128, 128], F32, tag="pA")
                for d in range(DCH):
                    nc.tensor.matmul(
                        ps[:], xts[d][:, t * KB:(t + 1) * KB], wv_sb[d][:],
                        start=(d == 0), stop=(d == DCH - 1))
                nc.vector.tensor_copy(
                    vt[:, t, :].rearrange("p (h c) -> p h c", c=65)[:, :, 0:64],
                    ps[:].rearrange("p (h c) -> p h c", c=64))
            v_t.append(vt)

        # ---- phase B: attention per (b, local head) ----
        o_all = pO.tile([128, R], BF, tag="oall")
        for b in range(B):
            for h2 in range(HPC):
                p0 = h2 * HD
                for qt_i in range(NQT):
                    q0 = qt_i * QT
                    po = ppo.tile([HD + 1, QT], F32, tag="po")
                    nkb = (q0 + QT) // KB
                    for kbi in range(nkb):
                        k0 = kbi * KB
                        ps = pps.tile([128, QT], F32, tag="ps")
                        nc.tensor.matmul(
                            ps[:],
                            kt_t[b][p0:p0 + HD, k0:k0 + KB],
                            qt_t[b][p0:p0 + HD, q0:q0 + QT],
                            start=True, stop=True)
                        pt = ppt.tile([128, QT], BF, tag="pt")
                        nc.scalar.activation(
                            pt[:], ps[:], mybir.ActivationFunctionType.Exp,
                            scale=float(HD) ** -0.5)
                        if k0 >= q0:  # diagonal block: causal mask
                            j = (k0 - q0) // KB
                            nc.vector.tensor_tensor(
                                pt[:], pt[:], mask_sb[j][:],
                                mybir.AluOpType.mult)
                        nc.tensor.matmul(
                            po[:],
                            v_t[b][:, kbi, :].rearrange(
                                "p (h c) -> p h c", c=65)[:, h2, :],
                            pt[:],
                            start=(kbi == 0), stop=(kbi == nkb - 1))
                    # normalize: O / l  (l = row 64 of po)
                    recip = pnorm.tile([1, QT], F32, tag="recip")
                    nc.vector.reciprocal(recip[:], po[HD:HD + 1, :])
                    pb = ppb.tile([HD, QT], F32, tag="pb")
                    nc.tensor.matmul(pb[:], ones_sb[:], recip[:],
                                     start=True, stop=True)
                    bc = pnorm.tile([HD, QT], F32, tag="bc")
                    nc.scalar.copy(bc[:], pb[:])
                    nc.vector.tensor_tensor(
                        o_all[p0:p0 + HD, b * S + q0: b * S + q0 + QT],
                        po[0:HD, :], bc[:], mybir.AluOpType.mult)

        # ---- phase C: reshard heads -> rows ----
        for j in range(NC):
            nc.sync.dma_start(out=a2a_in[j, :, :],
                              in_=o_all[:, j * RPC:(j + 1) * RPC])
        nc.gpsimd.collective_compute(
            "AllToAll", mybir.AluOpType.bypass,
            replica_groups=[list(range(NC))],
            ins=[a2a_in[:].opt()], outs=[a2a_out[:].opt()])

        # ---- phase D: output projection on my rows ----
        og = []
        for d in range(NC):
            t = pog.tile([128, RPC], BF, tag="og")
            nc.sync.dma_start(out=t[:], in_=a2a_out[d, :, :])
            og.append(t)
        for e in range(DCH):
            ps = ppA.tile([128, RPC], F32, tag="pA")
            for d in range(NC):
                nc.tensor.matmul(
                    ps[:], wo_sb[d][:, e * 128:(e + 1) * 128], og[d][:],
                    start=(d == 0), stop=(d == NC - 1))
            osb = posb.tile([128, RPC], F32, tag="osb")
            nc.vector.tensor_copy(osb[:], ps[:])
            nc.sync.dma_start(out=out[e * 128:(e + 1) * 128, :], in_=osb[:])

    nc.compile()
    return nc


_NC_CACHE = {}


def _get_nc(S):
    if S not in _NC_CACHE:
        _NC_CACHE[S] = build_nc(S)
    return _NC_CACHE[S]


def make_in_maps(x, wq, wk, wv, wo):
    b, S, d = x.shape
    xT = np.ascontiguousarray(x.reshape(b * S, d).T).astype(BF16)
    woT = np.ascontiguousarray(wo.T).astype(BF16)

    # RoPE tables, transposed: partition p -> head-local dim p % 64
    inv = (1.0 / ROPE_BASE ** (np.arange(0, HD, 2, dtype=np.float64) / HD))
    t = np.arange(S, dtype=np.float64)
    fr = np.outer(t, inv)                      # [S, 32]
    emb = np.concatenate([fr, fr], axis=1)     # [S, 64]
    cos_t = np.cos(emb).T                      # [64, S]
    sin_t = np.sin(emb).T
    sgn = np.where(np.arange(HD) < HD // 2, -1.0, 1.0)[:, None]
    cosT = np.concatenate([cos_t, cos_t], axis=0).astype(BF16)       # [128,S]
    sinT = np.concatenate([sin_t * sgn, sin_t * sgn], axis=0).astype(BF16)

    qf = np.arange(QT)[None, :]
    pp = np.arange(128)[:, None]
    maskT = np.stack([(qf >= j * KB + pp) for j in range(4)]).astype(BF16)

    in_maps = []
    for c in range(NC):
        sl = slice(c * DPC, (c + 1) * DPC)
        in_maps.append({
            "xT": xT,
            "wqT": np.ascontiguousarray(wq[sl, :].T).astype(BF16),
            "wkT": np.ascontiguousarray(wk[sl, :].T).astype(BF16),
            "wvT": np.ascontiguousarray(wv[sl, :].T).astype(BF16),
            "woT": woT,
            "cosT": cosT,
            "sinT": sinT,
            "maskT": maskT,
        })
    return in_maps


def run(x, wq, wk, wv, wo, trace=False):
    b, S, d = x.shape
    nc = _get_nc(S)
    in_maps = make_in_maps(x, wq, wk, wv, wo)
    res = run_bass_kernel_spmd(nc, in_maps, core_ids=list(range(NC)),
                               trace=trace)
    outT = np.concatenate([res.results[c]["out"] for c in range(NC)], axis=1)
    full = np.ascontiguousarray(outT.T).reshape(b, S, d).astype(np.float32)
    return full, res


def kernel(x, wq, wk, wv, wo):
    full, _ = run(np.asarray(x), np.asarray(wq), np.asarray(wk),
                  np.asarray(wv), np.asarray(wo))
    return full


# revision 11
# speedup vs baseline: 1.0590x; 1.0590x over previous
"""Multi-head attention (RoPE + causal SDPA) on 8 Trainium2 NeuronCores.

Sharding: tensor-parallel over heads for QKV+attention (2 heads/core),
then an on-device AllToAll (one per batch element, so comm overlaps the
other batch's compute) reshards from head-split to row-split, and each
core computes its row slice of the output projection with the full wo.
Host side only slices/transposes/concatenates.

Everything is computed "transposed" (feature dims on SBUF partitions,
sequence on the free axis), so no transposes are needed on the hot path:
  Qt/Kt  (128=2*64 head dims, S)  = W_slice @ x.T   (lhsT=W_sliceT chunks)
  scores (128 kpos, <=1024 q)     lhsT=Kt block, rhs=Qt slice
  P.T    = exp(scores/8)          on ScalarE; softmax sum comes from an
                                  extra ones-column appended to V
  O.T    (64+1, 1024 q)           accumulated over k blocks in PSUM
  out.T  (1024 e, rows)           = woT chunks.T @ O_full.T
"""
import sys, os
if '/opt/trn_rl_repo' not in sys.path:
    sys.path.insert(0, '/opt/trn_rl_repo')
os.environ.setdefault('MYCRO_LOCAL_CACHE', '1')

from contextlib import ExitStack

import numpy as np
import ml_dtypes

import concourse.bass as bass
import concourse.tile as tile
from concourse import bacc, mybir
from concourse.bass_utils import run_bass_kernel_spmd
from concourse.masks import make_identity

BF16 = ml_dtypes.bfloat16
NC = 8           # cores
B = 2            # batch
D = 1024         # model dim
H = 16           # heads
HD = 64          # head dim
HPC = H // NC    # heads per core = 2
DPC = HPC * HD   # head dims per core = 128
ROPE_BASE = 10000.0
QT = 512         # projection tile / narrow attention tile
KB = 128         # k block (partition axis of score matmuls)

F32 = mybir.dt.float32
BF = mybir.dt.bfloat16
MULT = mybir.AluOpType.mult
ADD = mybir.AluOpType.add


def build_nc(S):
    """Build+compile the SPMD Bass module for sequence length S."""
    GW = min(1024, S)      # attention group width (q columns)
    RH = S // NC           # rows per core per batch half
    NST = S // QT          # 512-wide seq tiles per batch
    NVT = S // KB          # V tiles (of 128 kpos) per batch
    DCH = D // 128         # contraction chunks (= 8)
    NG = S // GW           # attention groups per (b,h)

    nc = bacc.Bacc(num_devices=NC)

    xT = nc.declare_dram_parameter("xT", [D, B * S], BF, isOutput=False)
    wqT = nc.declare_dram_parameter("wqT", [D, DPC], BF, isOutput=False)
    wkT = nc.declare_dram_parameter("wkT", [D, DPC], BF, isOutput=False)
    wvT = nc.declare_dram_parameter("wvT", [D, DPC], BF, isOutput=False)
    woT = nc.declare_dram_parameter("woT", [D, D], BF, isOutput=False)
    cosT = nc.declare_dram_parameter("cosT", [128, S], BF, isOutput=False)
    sinT = nc.declare_dram_parameter("sinT", [128, S], BF, isOutput=False)
    maskW = nc.declare_dram_parameter("maskW", [4, 128, GW], BF, isOutput=False)
    maskN = nc.declare_dram_parameter("maskN", [4, 128, QT], BF, isOutput=False)
    out = nc.declare_dram_parameter("out", [D, B * RH], F32, isOutput=True)

    a2a_in = [nc.dram_tensor(f"a2a_in{b}", [NC, 128, RH], BF) for b in range(B)]
    a2a_out = [nc.dram_tensor(f"a2a_out{b}", [NC, 128, RH], BF) for b in range(B)]

    ctx = ExitStack()
    with ctx:
        tc = ctx.enter_context(tile.TileContext(nc))

        consts = ctx.enter_context(tc.tile_pool(name="consts", bufs=1))
        xpool = ctx.enter_context(tc.tile_pool(name="x", bufs=12))
        pQt = ctx.enter_context(tc.tile_pool(name="qt", bufs=2))
        pKt = ctx.enter_context(tc.tile_pool(name="kt", bufs=2))
        pV = ctx.enter_context(tc.tile_pool(name="v", bufs=2))
        pO = ctx.enter_context(tc.tile_pool(name="oall", bufs=2))
        ptmp = ctx.enter_context(tc.tile_pool(name="tmp", bufs=3))
        ppt = ctx.enter_context(tc.tile_pool(name="pt", bufs=4))
        pnorm = ctx.enter_context(tc.tile_pool(name="norm", bufs=2))
        pog = ctx.enter_context(tc.tile_pool(name="og", bufs=2 * NC))
        posb = ctx.enter_context(tc.tile_pool(name="osb", bufs=2))

        # PSUM: 8 banks total.
        # ppA (1 slot, 1 bank): projections + out-proj
        # pps (2 slots x 2 banks): score blocks (128, GW)
        # ppo (1 slot, 2 banks): O.T accumulator (65, GW)
        # ppb (1 slot, 1 bank): bcast outer products + PE-transpose outputs
        ppA = ctx.enter_context(tc.tile_pool(name="ppA", bufs=1, space="PSUM"))
        pps = ctx.enter_context(tc.tile_pool(name="pps", bufs=2, space="PSUM"))
        ppo = ctx.enter_context(tc.tile_pool(name="ppo", bufs=1, space="PSUM"))
        ppb = ctx.enter_context(tc.tile_pool(name="ppb", bufs=1, space="PSUM"))

        # ---- constants into SBUF ----
        def load_w(dram, ncols):
            ts = []
            for d in range(DCH):
                t = consts.tile([128, ncols], BF, tag=f"w{dram.name}{d}")
                nc.sync.dma_start(out=t[:], in_=dram[d * 128:(d + 1) * 128, :])
                ts.append(t)
            return ts

        wq_sb = load_w(wqT, DPC)
        wk_sb = load_w(wkT, DPC)
        wv_sb = load_w(wvT, DPC)
        wo_sb = load_w(woT, D)

        cos_sb = consts.tile([128, S], BF, tag="cos")
        nc.sync.dma_start(out=cos_sb[:], in_=cosT[:, :])
        sin_sb = consts.tile([128, S], BF, tag="sin")
        nc.sync.dma_start(out=sin_sb[:], in_=sinT[:, :])
        maskw_sb, maskn_sb = [], []
        for j in range(4):
            m = consts.tile([128, GW], BF, tag=f"mw{j}")
            nc.sync.dma_start(out=m[:], in_=maskW[j, :, :])
            maskw_sb.append(m)
            m = consts.tile([128, QT], BF, tag=f"mn{j}")
            nc.sync.dma_start(out=m[:], in_=maskN[j, :, :])
            maskn_sb.append(m)
        ones_sb = consts.tile([1, HD], BF, tag="ones")
        nc.vector.memset(ones_sb[:], 1.0)
        ident = consts.tile([128, 128], BF, tag="ident")
        make_identity(nc, ident[:])

        # ---------------- phase builders ----------------
        qt_t, kt_t, v_t, o_t = {}, {}, {}, {}

        def phaseA(b):
            xts = []
            for d in range(DCH):
                t = xpool.tile([128, S], BF, tag="xt")
                nc.sync.dma_start(out=t[:], in_=xT[d * 128:(d + 1) * 128,
                                                   b * S:(b + 1) * S])
                xts.append(t)

            def project_rope(w_sb, dest_pool, tag):
                dest = dest_pool.tile([128, S], BF, tag=tag)
                for st in range(NST):
                    ps = ppA.tile([128, QT], F32, tag="pA")
                    for d in range(DCH):
                        nc.tensor.matmul(
                            ps[:], w_sb[d][:], xts[d][:, st * QT:(st + 1) * QT],
                            start=(d == 0), stop=(d == DCH - 1))
                    c0 = st * QT
                    raw = ptmp.tile([128, QT], BF, tag="raw")
                    nc.vector.tensor_copy(raw[:], ps[:])
                    tcos = ptmp.tile([128, QT], BF, tag="tcos")
                    nc.vector.tensor_tensor(
                        tcos[:], raw[:], cos_sb[:, c0:c0 + QT], MULT)
                    trot = ptmp.tile([128, QT], BF, tag="trot")
                    for g in range(4):
                        o0 = g * 32
                        i0 = o0 + 32 if g % 2 == 0 else o0 - 32
                        nc.vector.tensor_copy(
                            trot[o0:o0 + 32, :], raw[i0:i0 + 32, :])
                    nc.vector.tensor_tensor(
                        trot[:], trot[:], sin_sb[:, c0:c0 + QT], MULT)
                    nc.vector.tensor_tensor(
                        dest[:, c0:c0 + QT], tcos[:], trot[:], ADD)
                return dest

            qt_t[b] = project_rope(wq_sb, pQt, "qt")
            kt_t[b] = project_rope(wk_sb, pKt, "kt")

            # V: compute V.T (wv stationary, x.T moving), PE-transpose to
            # natural (kpos, hd) layout, append a ones column per head:
            # vt[:, t, 0:65] = [V_headA | 1], [:, t, 65:130] = [V_headB | 1].
            vt = pV.tile([128, NVT, 130], BF, tag="vt")
            nc.vector.memset(vt[:], 1.0)
            for st in range(NST):
                ps = ppA.tile([128, QT], F32, tag="pA")
                for d in range(DCH):
                    nc.tensor.matmul(
                        ps[:], wv_sb[d][:], xts[d][:, st * QT:(st + 1) * QT],
                        start=(d == 0), stop=(d == DCH - 1))
                vts = ptmp.tile([128, QT], BF, tag="vts")
                nc.vector.tensor_copy(vts[:], ps[:])
                for i in range(QT // 128):
                    ptr = ppb.tile([128, 128], BF, tag="pb")
                    nc.tensor.transpose(
                        ptr[:], vts[:, i * 128:(i + 1) * 128], ident[:])
                    t = st * (QT // 128) + i
                    nc.vector.tensor_copy(
                        vt[:, t, :].rearrange("p (h c) -> p h c", c=65)[:, :, 0:64],
                        ptr[:].rearrange("p (h c) -> p h c", c=64))
            v_t[b] = vt

        def vext(b, kb, h2):
            return v_t[b][:, kb, :].rearrange("p (h c) -> p h c", c=65)[:, h2, :]

        def phaseB(b, h2):
            """Attention for (batch b, local head h2) -> o_t[b]."""
            p0 = h2 * HD
            if b not in o_t:
                o_t[b] = pO.tile([128, S], BF, tag="ob", name="ob")
            ob = o_t[b]
            for g in range(NG):
                q0 = g * GW
                po = ppo.tile([HD + 1, GW], F32, tag="po")
                nkb = (q0 + GW) // KB
                for kb in range(nkb):
                    k0 = kb * KB
                    if GW > QT and k0 >= q0 + QT:
                        n0, nw = q0 + GW - QT, QT      # narrow tail block
                        mask = maskn_sb[(k0 - n0) // KB]
                    elif k0 >= q0:
                        n0, nw = q0, GW                 # wide diagonal block
                        mask = (maskw_sb if GW > QT else maskn_sb)[(k0 - q0) // KB]
                    else:
                        n0, nw = q0, GW                 # full block
                        mask = None
                    ps = pps.tile([128, GW], F32, tag="ps")
                    for h0 in range(0, nw, QT):
                        nc.tensor.matmul(
                            ps[:, h0:h0 + QT],
                            kt_t[b][p0:p0 + HD, k0:k0 + KB],
                            qt_t[b][p0:p0 + HD, n0 + h0:n0 + h0 + QT],
                            start=True, stop=True, skip_group_check=True)
                    pt = ppt.tile([128, GW], BF, tag="pt")
                    nc.scalar.activation(
                        pt[:, 0:nw], ps[:, 0:nw],
                        mybir.ActivationFunctionType.Exp,
                        scale=float(HD) ** -0.5)
                    if mask is not None:
                        nc.vector.tensor_tensor(
                            pt[:, 0:nw], pt[:, 0:nw], mask[:], MULT)
                    for h0 in range(0, nw, QT):
                        nc.tensor.matmul(
                            po[:, n0 - q0 + h0:n0 - q0 + h0 + QT],
                            vext(b, kb, h2), pt[:, h0:h0 + QT],
                            start=(kb == 0), stop=(kb == nkb - 1),
                            skip_group_check=True)
                # normalize: O/l, l = row HD of po
                # (reciprocal_approx_fast reading PSUM directly corrupts on
                # HW -- bounce l through SBUF first)
                lsb = pnorm.tile([1, GW], F32, tag="lsb", name="lsb")
                nc.vector.tensor_copy(lsb[:], po[HD:HD + 1, :])
                recip32 = pnorm.tile([1, GW], F32, tag="recip32", name="recip32")
                nc.vector.reciprocal_approx_fast(recip32[:], lsb[:])
                recip = pnorm.tile([1, GW], BF, tag="recip", name="recip")
                nc.gpsimd.tensor_copy(recip[:], recip32[:])
                for hh in range(GW // QT):
                    c0 = hh * QT
                    pb = ppb.tile([HD, QT], F32, tag="pb")
                    nc.tensor.matmul(pb[:], ones_sb[:],
                                     recip[:, c0:c0 + QT],
                                     start=True, stop=True)
                    bc = pnorm.tile([HD, QT], BF, tag="bc")
                    nc.vector.tensor_copy(bc[:], pb[:])
                    nc.vector.tensor_tensor(
                        ob[p0:p0 + HD, q0 + c0:q0 + c0 + QT],
                        po[0:HD, c0:c0 + QT], bc[:], MULT)

        def phaseC(b):
            """Reshard batch-b rows: heads-split -> row-split."""
            for j in range(NC):
                nc.sync.dma_start(out=a2a_in[b][j, :, :],
                                  in_=o_t[b][:, j * RH:(j + 1) * RH])
            nc.gpsimd.collective_compute(
                "AllToAll", mybir.AluOpType.bypass,
                replica_groups=[list(range(NC))],
                ins=[a2a_in[b][:].opt()], outs=[a2a_out[b][:].opt()])

        def phaseD(b):
            """Out-projection for my RH rows of batch b."""
            og = []
            for d in range(NC):
                t = pog.tile([128, RH], BF, tag="og")
                nc.sync.dma_start(out=t[:], in_=a2a_out[b][d, :, :])
                og.append(t)
            for e in range(DCH):
                ps = ppA.tile([128, RH], F32, tag="pA")
                for d in range(NC):
                    nc.tensor.matmul(
                        ps[:], wo_sb[d][:, e * 128:(e + 1) * 128], og[d][:],
                        start=(d == 0), stop=(d == NC - 1))
                osb = posb.tile([128, RH], F32, tag="osb")
                nc.vector.tensor_copy(osb[:], ps[:])
                nc.sync.dma_start(
                    out=out[e * 128:(e + 1) * 128, b * RH:(b + 1) * RH],
                    in_=osb[:])

        # ---------------- schedule ----------------
        phaseA(0)
        phaseA(1)
        phaseB(0, 0)
        phaseB(0, 1)
        phaseC(0)
        phaseD(0)
        phaseB(1, 0)
        phaseB(1, 1)
        phaseC(1)
        phaseD(1)

    nc.compile()
    return nc


_NC_CACHE = {}


def _get_nc(S):
    if S not in _NC_CACHE:
        _NC_CACHE[S] = build_nc(S)
    return _NC_CACHE[S]


def make_in_maps(x, wq, wk, wv, wo):
    b, S, d = x.shape
    GW = min(1024, S)
    xT = np.ascontiguousarray(x.reshape(b * S, d).T).astype(BF16)
    woT = np.ascontiguousarray(wo.T).astype(BF16)

    # RoPE tables, transposed: partition p -> head-local dim p % 64
    inv = (1.0 / ROPE_BASE ** (np.arange(0, HD, 2, dtype=np.float64) / HD))
    t = np.arange(S, dtype=np.float64)
    fr = np.outer(t, inv)                      # [S, 32]
    emb = np.concatenate([fr, fr], axis=1)     # [S, 64]
    cos_t = np.cos(emb).T                      # [64, S]
    sin_t = np.sin(emb).T
    sgn = np.where(np.arange(HD) < HD // 2, -1.0, 1.0)[:, None]
    cosT = np.concatenate([cos_t, cos_t], axis=0).astype(BF16)       # [128,S]
    sinT = np.concatenate([sin_t * sgn, sin_t * sgn], axis=0).astype(BF16)

    pp = np.arange(128)[:, None]
    qn = np.arange(QT)[None, :]
    maskN = np.stack([(qn >= j * KB + pp) for j in range(4)]).astype(BF16)
    qw = np.arange(GW)[None, :]
    maskW = np.stack([(qw >= j * KB + pp) for j in range(4)]).astype(BF16)

    in_maps = []
    for c in range(NC):
        sl = slice(c * DPC, (c + 1) * DPC)
        in_maps.append({
            "xT": xT,
            "wqT": np.ascontiguousarray(wq[sl, :].T).astype(BF16),
            "wkT": np.ascontiguousarray(wk[sl, :].T).astype(BF16),
            "wvT": np.ascontiguousarray(wv[sl, :].T).astype(BF16),
            "woT": woT,
            "cosT": cosT,
            "sinT": sinT,
            "maskW": maskW,
            "maskN": maskN,
        })
    return in_maps


def assemble(outs, S):
    """outs[c] = per-core (D, B*RH) out.T block -> full (B, S, D)."""
    RH = S // NC
    outT = np.empty((D, B * S), dtype=np.float32)
    for c in range(NC):
        o = np.asarray(outs[c])
        for bb in range(B):
            outT[:, bb * S + c * RH: bb * S + (c + 1) * RH] = \
                o[:, bb * RH:(bb + 1) * RH]
    return np.ascontiguousarray(outT.T).reshape(B, S, D).astype(np.float32)


def run(x, wq, wk, wv, wo, trace=False):
    b, S, d = x.shape
    nc = _get_nc(S)
    in_maps = make_in_maps(x, wq, wk, wv, wo)
    res = run_bass_kernel_spmd(nc, in_maps, core_ids=list(range(NC)),
                               trace=trace)
    full = assemble([res.results[c]["out"] for c in range(NC)], S)
    return full, res


def kernel(x, wq, wk, wv, wo):
    full, _ = run(np.asarray(x), np.asarray(wq), np.asarray(wk),
                  np.asarray(wv), np.asarray(wo))
    return full
